# revision 1
# baseline (speedup 1.0000x reference)
"""BarrierNet Trainium2 kernel: MLP (6->128->128x2 branches->heads) + closed-form QP.

Data-parallel over 8 cores (16384 samples each):
  - MLP in transposed layout (hidden on partitions, batch free), 8 chunks of
    2048; each layer = 4 bf16 N=512 matmuls into a manually-windowed PSUM
    tensor (2x2048 windows for fc1/fc2; a 3x1024 ring + 2x512 heads region
    for the merged wm1+wm2+heads phase, which interleaves the independent
    wm1/wm2 matmul streams to stay PE-bound instead of alternating
    ACT-bound and PE-bound phases).
  - tanh split across ACT and DVE: ACT drains cols [0, 1536) of each chunk
    with fused bias+tanh; the last 512 cols of the error-insensitive layers
    (fc1, fc21, fcm1) go DVE: bias-copy PSUM->SBUF bf16, then a clamped
    deg-5 odd-poly tanh (tensor_scalar ops at 4x, tensor_tensor at 2x).
    QP x-side precomputation runs on the otherwise-idle Pool engine.
  - Heads: per chunk, fc31+fc32 accumulate into ONE PSUM bank via
    tile_position col-groups (x31 rows 32m+0..2, z/2 rows 32m+3..4, the
    sigmoid's 0.5 folded into wh32 host-side); ACT/DVE drain to SBUF bf16,
    4 plain-2D DMAs store to DRAM scratch (split A=chunks 0-6 / B=chunk 7
    to decouple deps), and two gathers rebuild a [128, 5*128] sample-grid
    where the batched QP math runs.
  - sigmoid via tanh: 4*sig(z) = 2+2*tanh(z/2) -> single ACT table; the QP
    tail algebra is rewritten around t0,t1 = tanh(z/2) with all
    gather-independent terms (C1, D0) precomputed on Pool.
"""
import sys

sys.path.insert(0, "/opt/trn_rl_repo")

import numpy as np
import ml_dtypes

import concourse.bacc as bacc
import concourse.bass as bass
import concourse.mybir as mybir
import concourse.tile as tile
from concourse import bass_utils

FP = mybir.dt.float32
BF = mybir.dt.bfloat16
AF = mybir.ActivationFunctionType
OP = mybir.AluOpType
BF_NP = ml_dtypes.bfloat16

N_CORES = 8
B = 131072
NS = B // N_CORES          # samples per core
SC = 2048                  # chunk (one PSUM tile span)
H = 128
NF = 6
NIT = NS // SC
JH = NS // 128             # samples per partition in the QP sample grid
J3, J2, J6 = 3 * JH, 2 * JH, 6 * JH
FC = 17                    # fpack: 5 biases + 12 denorm scalars
QW = J3 + J2               # then b31 | b32/2 blocks

# tanh offload widths (cols of each 2048 chunk done by DVE poly)
X1, X2, X3 = 512, 512, 512
# extra fc21 cols done by Pool poly (DVE only does the PSUM->SBUF copy);
# fc21's consumer (wm1) is a full phase later, so Pool's slower rate hides
P2 = 0

# clamped deg-5 poly: tanh(x) ~ clamp(x*((g*x^2+d)^2+e), -1, 1)
PC2 = 0.008226487
PAL = -8.014924
PBE = 43.06224
PG = float(np.sqrt(PC2))
PD = float(PAL * np.sqrt(PC2))
PE2 = float(PBE * PC2)

_cache = {}


def build(ns=NS):
    nc = bacc.Bacc("TRN2", target_bir_lowering=False, debug=False)

    x_d = nc.dram_tensor("x", [ns, NF], FP, kind="ExternalInput")
    xt_d = nc.dram_tensor("xt", [NF, ns], BF, kind="ExternalInput")
    w1T_d = nc.dram_tensor("w1T", [NF, H], BF, kind="ExternalInput")
    wpack_d = nc.dram_tensor("wpack", [H, 576 + J3 + J2], BF, kind="ExternalInput")
    fpack_d = nc.dram_tensor("fpack", [H, FC], FP, kind="ExternalInput")
    u_d = nc.dram_tensor("u", [ns, 3], FP, kind="ExternalOutput")

    with tile.TileContext(nc) as tc:
        with (
            tc.tile_pool(name="const", bufs=1) as cpool,
            tc.tile_pool(name="act", bufs=2) as apool,
            tc.tile_pool(name="xb", bufs=4) as xpool,
            tc.tile_pool(name="hd", bufs=3) as hpool,
            tc.tile_pool(name="psum", bufs=1, space="PSUM") as ppool,
            tc.tile_pool(name="qp", bufs=1) as qpool,
            tc.tile_pool(name="qtmp", bufs=1) as tpool,
            tc.tile_pool(name="dram", bufs=1, space="DRAM") as dpool,
        ):
            # split scratch: A = chunks 0-6, B = chunk 7, so the early
            # gather has no (false) dep on the last chunk's stores
            scrA = dpool.tile([32, 7 * SC], BF, tag="scrA", name="scrA")
            scrB = dpool.tile([32, SC], BF, tag="scrB", name="scrB")
            # ---------------- loads (all SP/HWDGE) --------------------------
            # dummy activation on a memset tile: pulls the ACT table load off
            # the critical path (runs at t~0, before any DMA lands)
            dum = cpool.tile([1, 2], FP, tag="dum", name="dum")
            nc.gpsimd.memset(dum[:], 0.0)
            nc.scalar.activation(dum[:], dum[:], AF.Tanh)
            def load_xtc(k):
                t = apool.tile([NF, 4096], BF, tag="xtc", name="xtc")
                nc.sync.dma_start(t[:], xt_d[:, 4096 * k : 4096 * (k + 1)])
                return t

            w1T = cpool.tile([NF, H], BF, tag="w1T", name="w1T")
            nc.sync.dma_start(w1T[:], w1T_d[:])
            xtc = {0: load_xtc(0)}
            fpack = cpool.tile([H, FC], FP, tag="fpack", name="fpack")
            nc.sync.dma_start(fpack[:, 0:5], fpack_d[:, 0:5])
            wpack = cpool.tile([H, 576 + J3 + J2], BF, tag="wpack", name="wpack")
            nc.sync.dma_start(wpack[:], wpack_d[:])
            nc.sync.dma_start(fpack[:, 5:], fpack_d[:, 5:])
            xtc[1] = load_xtc(1)

            w21T = wpack[:, 0:128]
            w22T = wpack[:, 128:256]
            wm1T = wpack[:, 256:384]
            wm2T = wpack[:, 384:512]
            whp31 = wpack[:, 512:544]
            whp32 = wpack[:, 544:576]
            b1, b21, b22, bm1, bm2 = (fpack[:, i : i + 1] for i in range(5))
            qb31 = wpack[:, 576 : 576 + J3]
            qb32 = wpack[:, 576 + J3 : 576 + J3 + J2]

            xg = cpool.tile([128, J6], FP, tag="xg", name="xg")
            nc.sync.dma_start(
                xg[:], x_d[:].rearrange("(p j) f -> p (j f)", p=128))

            gA = cpool.tile([128, 5 * JH], BF, tag="gA", name="gA")

            # ---------------- QP (sample-grid layout, fp32, batched) --------
            QS = {}

            def T(tag, w):
                t = tpool.tile([128, w], FP, tag=tag, name=tag)
                QS[tag] = t[:]
                return t[:]

            def qp_pre():
                V = nc.gpsimd    # all-SBUF: legal on Pool, frees DVE
                xgv = xg.rearrange("p (j g e) -> p e g j", g=3, e=2)
                x0 = T("x0", J6)
                x0v = x0.rearrange("p (e g j) -> p e g j", e=2, g=3)
                for e in range(2):
                    for g_ in range(3):
                        sd = fpack[:, 5 + 2 * (3 * e + g_) : 6 + 2 * (3 * e + g_)]
                        mo = fpack[:, 6 + 2 * (3 * e + g_) : 7 + 2 * (3 * e + g_)]
                        V.tensor_scalar(x0v[:, e, g_, :], xgv[:, e, g_, :],
                                        sd, mo, OP.mult, OP.add)
                dd, vv = x0[:, 0:J3], x0[:, J3:J6]
                s1 = T("s1", J3); V.tensor_mul(s1, dd, dd)        # d^2
                d3 = T("d3", J3); V.tensor_mul(d3, s1, dd)

                def a3(t, k):
                    return t[:, k * JH : (k + 1) * JH]

                def sum3(t, tag, bias_const=None):
                    r = T(tag, JH)
                    V.tensor_add(r, a3(t, 0), a3(t, 1))
                    V.tensor_add(r, r, a3(t, 2))
                    if bias_const is not None:
                        V.tensor_scalar(r, r, bias_const, None, OP.add)
                    return r

                s2 = T("s2", J3); V.tensor_mul(s2, s1, s1)        # d^4
                bar = sum3(s2, "bar", -2401.0)   # barrier
                V.tensor_mul(s2, vv, vv)                          # v^2
                s3 = T("s3", J3); V.tensor_mul(s3, s1, s2)        # d^2 v^2
                Ls = sum3(s3, "Ls")              # Lf2b / 12
                V.tensor_mul(s3, d3, vv)                          # d^3 v
                bd = sum3(s3, "bd")              # barrier_dot / 4
                V.tensor_mul(s3, d3, d3)                          # d^6
                g6 = sum3(s3, "g6")              # GG / 16
                rg = T("rg", JH); nc.vector.reciprocal(rg, g6)
                # tail-chain precomputes:
                # lamnum = 4gu + C1 - 4S*D0 - 4P*bar,  D0 = 2bd+bar,
                # C1 = -12Ls - 16bd - 4bar
                D0 = T("D0", JH)
                V.tensor_scalar(D0, bd, 2.0, None, OP.mult)
                V.tensor_add(D0, D0, bar)
                C1 = T("C1", JH)
                V.tensor_scalar(C1, Ls, -12.0, None, OP.mult)
                sc_ = T("sc_", JH)
                V.tensor_scalar(sc_, bd, -16.0, None, OP.mult)
                V.tensor_add(C1, C1, sc_)
                V.tensor_scalar(sc_, bar, -4.0, None, OP.mult)
                V.tensor_add(C1, C1, sc_)

            def qp_post():
                V = nc.vector   # STT not supported on Pool
                q_ = QS
                zs = tpool.tile([128, J2], BF, tag="zs", name="zs")[:]
                V.tensor_add(zs, gA[:, J3 : J3 + J2], qb32)
                tt = tpool.tile([128, J2], BF, tag="tt", name="tt")[:]
                nc.scalar.activation(tt, zs, AF.Tanh)   # t = tanh(z/2)
                x31v = tpool.tile([128, J3], BF, tag="x31v", name="x31v")[:]
                V.tensor_add(x31v, gA[:, 0:J3], qb31)
                d3 = q_["d3"]
                gx = T("gx", J3); V.tensor_mul(gx, d3, x31v)
                gu = T("gu", JH)
                V.tensor_add(gu, gx[:, 0:JH], gx[:, JH : 2 * JH])
                V.tensor_add(gu, gu, gx[:, 2 * JH : J3])
                C0 = T("C0", JH)
                V.scalar_tensor_tensor(C0, gu, 4.0, q_["C1"], OP.mult, OP.add)
                t0, t1 = tt[:, 0:JH], tt[:, JH:J2]
                S = T("S", JH); V.tensor_add(S, t0, t1)
                P = T("P", JH); V.tensor_mul(P, t0, t1)
                e1 = T("e1", JH)
                V.scalar_tensor_tensor(e1, S, -4.0, q_["D0"], OP.mult, OP.mult)
                e2 = T("e2", JH)
                V.scalar_tensor_tensor(e2, P, -4.0, q_["bar"], OP.mult, OP.mult)
                q = T("q", JH)
                V.tensor_add(q, C0, e1)
                V.tensor_add(q, q, e2)
                lam = T("lam", JH)
                V.scalar_tensor_tensor(lam, q, 0.0, q_["rg"], OP.max, OP.mult)
                ui = qpool.tile([128, 3 * JH], FP, tag="ui", name="ui")
                uiv = ui.rearrange("p (j c) -> p c j", c=3)
                lam3 = bass.AP(lam.tensor, lam.offset,
                               [lam.ap[0], [0, 3], [1, JH]])
                w3 = T("w3", J3)
                V.tensor_mul(w3.rearrange("p (c j) -> p c j", c=3), lam3,
                             d3.rearrange("p (c j) -> p c j", c=3))
                V.scalar_tensor_tensor(uiv, w3.rearrange("p (c j) -> p c j", c=3),
                                       0.25,
                                       x31v.rearrange("p (c j) -> p c j", c=3),
                                       OP.mult, OP.subtract)
                nc.sync.dma_start(
                    u_d[:].rearrange("(p j) c -> p (j c)", p=128), ui[:])

            # ---------------- PSUM: one manually-windowed tensor ------------
            # fc1/fc2/wm1: alternating [0:2048)/[2048:4096) windows.
            # wm2: 1024-wide 3-slot ring over [0:3072); heads: 512-wide
            # 2-slot ring over [3072:4096). Subtile dep tracking keeps
            # disjoint windows independent.
            psall = ppool.tile([128, 4096], FP, tag="psall", name="psall")
            pcnt = [0]

            def poly(V, xb, out_sl, tag):
                """clamped deg-5 tanh poly; the square runs on Pool (its only
                PSUM-free step), the rest on engine V."""
                ta = xpool.tile([128, xb.shape[1]], BF, tag=tag + "a", name="ta")
                V.tensor_mul(ta[:], xb, xb)                       # s = x^2
                V.tensor_scalar(ta[:], ta[:], PG, PD, OP.mult, OP.add)
                V.tensor_mul(ta[:], ta[:], ta[:])                 # (gs+d)^2
                V.tensor_scalar(ta[:], ta[:], PE2, None, OP.add)
                tb_ = xpool.tile([128, xb.shape[1]], BF, tag=tag + "b", name="tb")
                V.tensor_mul(tb_[:], ta[:], xb)
                V.tensor_scalar(out_sl, tb_[:], 1.0, -1.0, OP.min, OP.max)

            # ---------------- MLP chunk helper ------------------------------
            def mlp_chunk(lhsT, rhs_sl, bias_ap, out_sl, xoff, width=SC,
                          poff=0):
                w0 = (pcnt[0] % 2) * 2048 if width == SC else 1024 * (pcnt[0] % 3)
                pcnt[0] += 1
                ps = psall[:, w0 : w0 + width]
                for m in range(width // 512):
                    nc.tensor.matmul(
                        ps[:, 512 * m : 512 * (m + 1)],
                        lhsT,
                        rhs_sl[:, 512 * m : 512 * (m + 1)],
                        start=True, stop=True,
                    )
                keep = width - xoff - poff
                nc.scalar.activation(out_sl[:, 0:keep], ps[:, 0:keep],
                                     AF.Tanh, bias=bias_ap)
                if xoff:
                    xb = xpool.tile([128, xoff], BF, tag="xb", name="xb")
                    nc.vector.tensor_scalar(xb[:], ps[:, keep : keep + xoff],
                                            bias_ap, None, OP.add)
                    poly(nc.vector, xb[:], out_sl[:, keep : keep + xoff], "v")
                if poff:
                    xp = xpool.tile([128, poff], BF, tag="xp", name="xp")
                    nc.vector.tensor_scalar(xp[:], ps[:, width - poff : width],
                                            bias_ap, None, OP.add)
                    poly(nc.gpsimd, xp[:], out_sl[:, width - poff : width], "p")

            def csl(t, i):
                return t[:, SC * i : SC * (i + 1)]

            # ---------------- layers ---------------------------------------
            hT_all = cpool.tile([H, ns], BF, tag="hT_all", name="hT_all")
            x21a = cpool.tile([H, ns], BF, tag="x21a", name="x21a")
            x22a = cpool.tile([H, ns], BF, tag="x22a", name="x22a")
            x21b = cpool.tile([H, ns], BF, tag="x21b", name="x21b")

            for i in range(NIT):
                if i in (2, 4) and i // 2 + 1 < 4:
                    xtc[i // 2 + 1] = load_xtc(i // 2 + 1)
                rhs = xtc[i // 2][:, 2048 * (i % 2) : 2048 * (i % 2 + 1)]
                mlp_chunk(w1T[:], rhs, b1, csl(hT_all, i), X1)
            qp_pre()
            for i in range(NIT):
                mlp_chunk(w21T, csl(hT_all, i), b21, csl(x21a, i), X2, poff=P2)
                mlp_chunk(w22T, csl(hT_all, i), b22, csl(x22a, i), 0)
            def heads(j):
                hw0 = 3072 + 512 * (j % 2)
                psH = psall[:, hw0 : hw0 + 512]
                x21 = csl(x21b, j)
                x22 = x22t[j]
                for m in range(4):
                    nc.tensor.matmul(psH[32 * m : 32 * m + 32, :], whp31,
                                     x21[:, 512 * m : 512 * (m + 1)],
                                     start=True, stop=False,
                                     tile_position=(0, 32 * m))
                    nc.tensor.matmul(psH[32 * m : 32 * m + 32, :], whp32,
                                     x22[:][:, 512 * m : 512 * (m + 1)],
                                     start=False, stop=True,
                                     tile_position=(0, 32 * m))
                hdp = hpool.tile([128, 512], BF, tag="hdp", name="hdp")
                tailc = j >= NIT - 2
                if tailc:
                    # tail-critical: drain on ACT (identity) so the stores
                    # don't queue behind DVE work
                    nc.scalar.activation(hdp[:], psH[:], AF.Identity)
                else:
                    nc.vector.tensor_copy(hdp[:], psH[:])
                scr = scrB if j == NIT - 1 else scrA
                col0 = 0 if j == NIT - 1 else 2048 * j
                for m in range(4):
                    eng = nc.scalar if (j == NIT - 1 and m % 2 == 1) else nc.sync
                    eng.dma_start(
                        scr[:, col0 + 512 * m : col0 + 512 * (m + 1)],
                        hdp[32 * m : 32 * m + 32, :])
                # early gather from Pool SWDGE: its sem-wait must not stall
                # SP's store pipeline; late gather on SP (nothing queued after)
                if j == NIT - 2:   # early gather: chunks 0-6
                    nc.gpsimd.dma_start(
                        gA[0:112, :].rearrange("p (c j) -> p c j", c=5),
                        scrA[0:5, :].rearrange("c (p j) -> p c j", p=112))
                if j == NIT - 1:   # late gather: last chunk only
                    nc.sync.dma_start(
                        gA[112:128, :].rearrange("p (c j) -> p c j", c=5),
                        scrB[0:5, :].rearrange("c (p j) -> p c j", p=16))

            # merged wm1+wm2 phase: wm2 depends only on fc22, so its
            # matmuls interleave with wm1's on the 1024-window ring; the
            # phase runs PE-bound (~3.4us/chunk) instead of an ACT-bound
            # wm1 phase followed by a PE-bound wm2 phase (~4us/chunk).
            x22t = {}
            pcnt[0] = 0
            for i in range(NIT):
                if i > 0 and i - 1 >= NIT - 2:
                    heads(i - 1)      # tail chunks: ACT-drain ahead of fcm2
                for h2 in range(2):
                    mlp_chunk(wm1T,
                              csl(x21a, i)[:, 1024 * h2 : 1024 * (h2 + 1)],
                              bm1, csl(x21b, i)[:, 1024 * h2 : 1024 * (h2 + 1)],
                              X3 if h2 == 1 else 0, width=1024)
                x22 = apool.tile([H, SC], BF, tag="x22b", name="x22b", bufs=3)
                for h2 in range(2):
                    mlp_chunk(wm2T,
                              csl(x22a, i)[:, 1024 * h2 : 1024 * (h2 + 1)],
                              bm2, x22[:, 1024 * h2 : 1024 * (h2 + 1)], 0,
                              width=1024)
                x22t[i] = x22
                if i > 0 and i - 1 < NIT - 2:
                    heads(i - 1)
            heads(NIT - 1)
            qp_post()

    nc.compile()
    return nc


def _get_nc(ns=NS):
    if ns not in _cache:
        _cache[ns] = build(ns)
    return _cache[ns]


def prep_maps(inputs, ns=NS, n_cores=N_CORES):
    """Host-side shard + layout prep. Returns per-core in_maps."""
    f32 = np.float32
    jh = ns // 128
    g = {k: np.asarray(v) for k, v in inputs.items()}
    x = np.ascontiguousarray(g["x"], f32)
    mean = np.asarray(g["mean"], f32)
    std = np.asarray(g["std"], f32)
    obs = np.array([10.0, 0.0, 10.0, 0.0, 9.0, 0.0], f32)
    moff = mean - obs
    perm = [0, 2, 4, 1, 3, 5]  # pos-block | vel-block order
    # denorm scalars interleaved std/moff in perm order (12 cols)
    dsc = np.empty(12, f32)
    dsc[0::2] = std[perm]
    dsc[1::2] = moff[perm]
    qc = dsc
    qb = np.concatenate([
        np.repeat(np.asarray(g["fc31_b"], f32), jh),
        np.repeat(np.asarray(g["fc32_b"], f32) * 0.5, jh),
    ])

    def padT(w, cols, scale=1.0, row0=0):
        out = np.zeros((H, 32), f32)
        out[:, row0 : row0 + cols] = np.asarray(w, f32).T * scale
        return out

    wpack = np.concatenate([
        np.asarray(g["fc21_w"], f32).T,
        np.asarray(g["fc22_w"], f32).T,
        np.asarray(g["fcm1_w"], f32).T,
        np.asarray(g["fcm2_w"], f32).T,
        padT(g["fc31_w"], 3),
        padT(g["fc32_w"], 2, scale=0.5, row0=3),
        np.broadcast_to(qb, (H, qb.size)),
    ], axis=1)
    wpack = np.ascontiguousarray(wpack.astype(BF_NP))

    fpack = np.concatenate([
        np.stack([np.asarray(g[k], f32) for k in
                  ("fc1_b", "fc21_b", "fc22_b", "fcm1_b", "fcm2_b")], axis=1),
        np.broadcast_to(qc, (H, qc.size)),
    ], axis=1)
    fpack = np.ascontiguousarray(fpack, f32)

    shared = {
        "w1T": np.ascontiguousarray(np.asarray(g["fc1_w"], f32).T.astype(BF_NP)),
        "wpack": wpack,
        "fpack": fpack,
    }
    in_maps = []
    for c in range(n_cores):
        sh = x[c * ns : (c + 1) * ns]
        m = dict(shared)
        m["x"] = np.ascontiguousarray(sh)
        m["xt"] = np.ascontiguousarray(sh.T.astype(BF_NP))
        in_maps.append(m)
    return in_maps


def kernel(**inputs):
    nc = _get_nc()
    in_maps = prep_maps(inputs)
    res = bass_utils.run_bass_kernel_spmd(nc, in_maps, core_ids=list(range(N_CORES)))
    return np.concatenate([res.results[c]["u"] for c in range(N_CORES)], axis=0)

